# revision 47
# baseline (speedup 1.0000x reference)
"""Trainium2 Bass kernel for nn_LogicLayer (difflogic LogicLayer forward).

Computation (reference):
    w  = softmax(weights, axis=-1)            # [OUT, 16]
    c  = w @ GATE_M                           # [OUT, 4]
    a  = x[:, idx_a]; b = x[:, idx_b]         # [B, OUT] feature gathers
    out = c0 + c1*a + c2*b + c3*(a*b)

Strategy (8 NeuronCores, feature-parallel, fp16/u8 data plane):
  - x is uploaded transposed twice — as fp16 (xT, a-row gathers) and as
    uint8 q=rint(255x) (xTb, b-row gathers) — and replicated; each core
    computes OUT/8 = 2048 output features over the full batch.
    Per-core HBM traffic: 16.8 MiB a-gather + 8.4 MiB b-gather +
    17 MiB fp16 output = 42 MiB -> ~118 us roofline at 358 GB/s.
    Quantization (fp16 a, u8 b, fp16 out) gives rel err ~2.2e-3,
    well under the 2e-2 gate.
  - Per 128-feature chunk, one dma_gather per stream (a, b) pulls the
    needed xT rows from HBM by int16 index — one descriptor per row.
    All 32 calls are emitted before anything else (4 SWDGE queues,
    deep tile pools) so the SDMA stream runs back-to-back.
  - Gate coefficients c0..c3 are computed on-device from `weights`
    (exp on ScalarE, strided-AP reductions + small tensor ops on
    VectorE); c2q/c3q = c{2,3}/255 pre-fold the u8 dequant.
  - Per chunk out = c0 + c1*a + c2*b + c3*a*b is evaluated two ways to
    balance ScalarE and VectorE (see main-loop comment): affines from
    the u8 b-rows fold the dequant into ScalarE activations for free;
    affines from the fp16 a-rows run as 4x VectorE tensor_scalar;
    the two tensor_tensor FMA passes always run on VectorE (2x mode).
  - Output written as outT [2048, B] fp16 (contiguous 8 KB per
    partition); host unshard transposes back and upcasts.
"""

import numpy as np

BATCH, IN_DIM, OUT_DIM = 4096, 16384, 16384
N_CORES = 8
F_CORE = OUT_DIM // N_CORES  # 2048 output features per core
P = 128


def _build_nc(in_dim, feat_core, batch):
    """Build + compile the per-core Bass program (SPMD, identical cores)."""
    from contextlib import ExitStack

    import concourse.bacc as bacc
    import concourse.mybir as mybir
    import concourse.tile as tile

    F32 = mybir.dt.float32
    F16 = mybir.dt.float16
    I16 = mybir.dt.int16
    TT = feat_core // P  # feature chunks per core (16)
    mult = mybir.AluOpType.mult
    add = mybir.AluOpType.add
    subtract = mybir.AluOpType.subtract
    Ident = mybir.ActivationFunctionType.Identity

    nc = bacc.Bacc(
        "TRN2", target_bir_lowering=False, debug=False, num_swdge_queues=4
    )
    U8 = mybir.dt.uint8
    Copy = mybir.ActivationFunctionType.Copy
    xT = nc.dram_tensor("xT", [in_dim, batch], F16, kind="ExternalInput")
    xTb = nc.dram_tensor("xTb", [in_dim, batch], U8, kind="ExternalInput")
    w = nc.dram_tensor("w", [feat_core, 16], F32, kind="ExternalInput")
    # separate gather indices for the a-rows (fp16) and b-rows (u8)
    idxA = nc.dram_tensor("idxA", [P, feat_core // 16], I16, kind="ExternalInput")
    idxB = nc.dram_tensor("idxB", [P, feat_core // 16], I16, kind="ExternalInput")
    outT = nc.dram_tensor("outT", [feat_core, batch], F16, kind="ExternalOutput")

    with tile.TileContext(nc) as tc, ExitStack() as ctx:
        const_pool = ctx.enter_context(tc.tile_pool(name="const", bufs=1))
        ga_pool = ctx.enter_context(tc.tile_pool(name="ga", bufs=7))
        gb_pool = ctx.enter_context(tc.tile_pool(name="gb", bufs=6))
        uv_pool = ctx.enter_context(tc.tile_pool(name="uv", bufs=5))

        idxA_sb = const_pool.tile([P, feat_core // 16], I16, tag="idxA")
        nc.sync.dma_start(idxA_sb[:], idxA[:])
        idxB_sb = const_pool.tile([P, feat_core // 16], I16, tag="idxB")
        nc.sync.dma_start(idxB_sb[:], idxB[:])

        c0 = const_pool.tile([P, TT], F32, tag="c0")
        c1 = const_pool.tile([P, TT], F32, tag="c1")
        c2 = const_pool.tile([P, TT], F32, tag="c2")
        c3 = const_pool.tile([P, TT], F32, tag="c3")
        c2q = const_pool.tile([P, TT], F32, tag="c2q")
        c3q = const_pool.tile([P, TT], F32, tag="c3q")

        # ---------- issue all gathers first ----------
        # Gathers depend only on the index tiles; emitting them before the
        # coefficient setup lets the SWDGE/SDMA pipeline start immediately.
        # One call per 128-row feature-chunk per stream (small calls keep
        # the Q7 head-of-line stall quantum low and the SDMA stream
        # smooth). a-rows come from the fp16 copy of xT, b-rows from the
        # uint8 copy (half the read bytes; dequantized on ScalarE).
        ga_tiles, gb_tiles = [], []
        for ci in range(TT):
            ga_t = ga_pool.tile([P, 1, batch], F16, tag="ga")
            nc.gpsimd.dma_gather(
                ga_t[:], xT[:], idxA_sb[:, ci * 8 : (ci + 1) * 8], 128, 128,
                batch, queue_num=(2 * ci) % 4,
            )
            ga_tiles.append(ga_t)
            gb_t = gb_pool.tile([P, 1, batch], U8, tag="gb")
            nc.gpsimd.dma_gather(
                gb_t[:], xTb[:], idxB_sb[:, ci * 8 : (ci + 1) * 8], 128, 128,
                batch, queue_num=(2 * ci + 1) % 4,
            )
            gb_tiles.append(gb_t)

        # ---------- gate coefficients (tiny; lives in const pool) ----------
        if True:
            sp = const_pool
            w_sb = sp.tile([P, TT, 16], F32, tag="wsb")
            nc.sync.dma_start(w_sb[:], w[:].rearrange("(t p) g -> p t g", p=P))
            E = sp.tile([P, TT, 16], F32, tag="E")
            nc.scalar.activation(E[:], w_sb[:], mybir.ActivationFunctionType.Exp)

            su = sp.tile([P, TT], F32, tag="su")
            nc.vector.reduce_sum(su[:], E[:], axis=mybir.AxisListType.X)
            r = sp.tile([P, TT], F32, tag="r")
            nc.vector.reciprocal(r[:], su[:])

            c0u = sp.tile([P, TT], F32, tag="c0u")
            nc.vector.reduce_sum(c0u[:], E[:, :, 8:16], axis=mybir.AxisListType.X)

            E4 = E[:].rearrange("p t (g2 g1) -> p t g2 g1", g1=4)
            a1 = sp.tile([P, TT], F32, tag="a1")
            nc.vector.reduce_sum(a1[:], E4[:, :, 0:2, 2:4], axis=mybir.AxisListType.XY)
            b1 = sp.tile([P, TT], F32, tag="b1")
            nc.vector.reduce_sum(b1[:], E4[:, :, 2:4, 0:2], axis=mybir.AxisListType.XY)
            c1u = sp.tile([P, TT], F32, tag="c1u")
            nc.vector.tensor_tensor(c1u[:], a1[:], b1[:], op=subtract)

            a2 = sp.tile([P, TT], F32, tag="a2")
            nc.vector.reduce_sum(a2[:], E[:, :, 4:8], axis=mybir.AxisListType.X)
            b2 = sp.tile([P, TT], F32, tag="b2")
            nc.vector.reduce_sum(b2[:], E[:, :, 8:12], axis=mybir.AxisListType.X)
            c2u = sp.tile([P, TT], F32, tag="c2u")
            nc.vector.tensor_tensor(c2u[:], a2[:], b2[:], op=subtract)

            # c3 = (E1+E8) + (E11+E13) - (E2+E4) - (E7+E14) - 2*(E6-E9)
            def eg(g):
                return E[:, :, g : g + 1]

            p1 = sp.tile([P, TT, 1], F32, tag="p1")
            nc.vector.tensor_tensor(p1[:], eg(1), eg(8), op=add)
            p2 = sp.tile([P, TT, 1], F32, tag="p2")
            nc.vector.tensor_tensor(p2[:], eg(11), eg(13), op=add)
            n1 = sp.tile([P, TT, 1], F32, tag="n1")
            nc.vector.tensor_tensor(n1[:], eg(2), eg(4), op=add)
            n2 = sp.tile([P, TT, 1], F32, tag="n2")
            nc.vector.tensor_tensor(n2[:], eg(7), eg(14), op=add)
            d6 = sp.tile([P, TT, 1], F32, tag="d6")
            nc.vector.tensor_tensor(d6[:], eg(6), eg(9), op=subtract)
            pp = sp.tile([P, TT, 1], F32, tag="pp")
            nc.vector.tensor_tensor(pp[:], p1[:], p2[:], op=add)
            nn_ = sp.tile([P, TT, 1], F32, tag="nn")
            nc.vector.tensor_tensor(nn_[:], n1[:], n2[:], op=add)
            c3a = sp.tile([P, TT, 1], F32, tag="c3a")
            nc.vector.tensor_tensor(c3a[:], pp[:], nn_[:], op=subtract)
            c3u = sp.tile([P, TT, 1], F32, tag="c3u")
            nc.vector.scalar_tensor_tensor(
                c3u[:], d6[:], -2.0, c3a[:], op0=mult, op1=add
            )

            nc.vector.tensor_tensor(c0[:], c0u[:], r[:], op=mult)
            nc.vector.tensor_tensor(c1[:], c1u[:], r[:], op=mult)
            nc.vector.tensor_tensor(c2[:], c2u[:], r[:], op=mult)
            nc.vector.tensor_tensor(c3[:], c3u[:, :, 0], r[:], op=mult)
            # u8-dequant-folded copies: c{2,3}/255 for the b-side affines
            nc.vector.tensor_scalar(
                c2q[:], c2[:], 1.0 / 255.0, None, op0=mult
            )
            nc.vector.tensor_scalar(
                c3q[:], c3[:], 1.0 / 255.0, None, op0=mult
            )

        # ---------- main FMA loop ----------
        # out = c0 + c1*a + c2*b + c3*a*b, evaluated two ways to balance
        # ScalarE and VectorE:
        #   even chunks:  out = (c1 + c3*b)*a + (c0 + c2*b)
        #     both affines on ScalarE directly from the u8 b-rows (the
        #     1/255 dequant folds into the activation scale for free);
        #     VectorE does only the two tensor_tensor passes.
        #   odd chunks:   out = (c2 + c3*a)*b + (c0 + c1*a)
        #     affines from the fp16 a-rows on VectorE (4x tensor_scalar),
        #     ScalarE does only the b dequant.
        # Totals: S ~ 8*(2*3.6) + 8*3.6 = 86us, V ~ 16*4.5 + 8*2.7 = 95us.
        for ci in range(TT):
            a_v = ga_tiles[ci][:, 0, :]
            qb_v = gb_tiles[ci][:, 0, :]
            cs = slice(ci, ci + 1)
            v = uv_pool.tile([P, batch], F16, tag="v")
            u = uv_pool.tile([P, batch], F16, tag="u")
            if ci % 2 == 0:
                # z = c1 + c3*(qb/255) ; w = c0 + c2*(qb/255)  (ScalarE)
                nc.scalar.activation(
                    v[:], qb_v, Ident, bias=c1[:, cs], scale=c3q[:, cs]
                )
                nc.scalar.activation(
                    u[:], qb_v, Ident, bias=c0[:, cs], scale=c2q[:, cs]
                )
                # v = v*a + u  (VectorE)
                nc.vector.tensor_tensor(v[:], v[:], a_v, op=mult)
                nc.vector.tensor_tensor(v[:], v[:], u[:], op=add)
            else:
                # b = qb/255 (ScalarE); v = c2 + c3*a, u = c0 + c1*a (VectorE)
                b16 = uv_pool.tile([P, batch], F16, tag="b16")
                nc.scalar.activation(
                    b16[:], qb_v, Copy, bias=0.0, scale=1.0 / 255.0
                )
                nc.vector.tensor_scalar(
                    v[:], a_v, c3[:, cs], c2[:, cs], op0=mult, op1=add
                )
                nc.vector.tensor_scalar(
                    u[:], a_v, c1[:, cs], c0[:, cs], op0=mult, op1=add
                )
                # v = v*b + u  (VectorE)
                nc.vector.tensor_tensor(v[:], v[:], b16[:], op=mult)
                nc.vector.tensor_tensor(v[:], v[:], u[:], op=add)
            nc.sync.dma_start(outT[ci * P : (ci + 1) * P, :], v[:])

    nc.compile()
    return nc


def _pack_idx(idx, feat_lo, feat_hi):
    """Host-side int16 gather-index buffer for one core (one stream).

    Per 128-feature chunk (one dma_gather call): indices in feature
    order. dma_gather consumes index i from partition i%16, column
    i//16 (replicated across the 8 groups of 16 partitions).
    """
    cols = []
    for f0 in range(feat_lo, feat_hi, P):
        ids = idx[f0 : f0 + P].astype(np.int16)
        blk = ids.reshape(8, 16)  # [col, partition-within-16]
        cols.append(np.tile(blk.T, (P // 16, 1)))  # [128, 8]
    return np.ascontiguousarray(np.concatenate(cols, axis=1))


_NC_CACHE = {}


def _get_nc():
    key = (IN_DIM, F_CORE, BATCH)
    if key not in _NC_CACHE:
        _NC_CACHE[key] = _build_nc(IN_DIM, F_CORE, BATCH)
    return _NC_CACHE[key]


TRACE = False  # set by dev harness to capture an NTFF profile
LAST_RESULT = None


def kernel(x, weights, idx_a, idx_b):
    global LAST_RESULT
    from concourse.bass_utils import run_bass_kernel_spmd

    x = np.asarray(x, dtype=np.float32)
    weights = np.asarray(weights, dtype=np.float32)
    idx_a = np.asarray(idx_a)
    idx_b = np.asarray(idx_b)

    nc = _get_nc()
    xT = np.ascontiguousarray(x.astype(np.float16).T)
    xTb = np.ascontiguousarray(np.rint(x * 255.0).astype(np.uint8).T)
    in_maps = []
    for k in range(N_CORES):
        lo, hi = k * F_CORE, (k + 1) * F_CORE
        in_maps.append(
            {
                "xT": xT,
                "xTb": xTb,
                "w": np.ascontiguousarray(weights[lo:hi]),
                "idxA": _pack_idx(idx_a, lo, hi),
                "idxB": _pack_idx(idx_b, lo, hi),
            }
        )

    res = run_bass_kernel_spmd(nc, in_maps, list(range(N_CORES)), trace=TRACE)
    LAST_RESULT = res
    out = np.empty((BATCH, OUT_DIM), dtype=np.float32)
    for k in range(N_CORES):
        out[:, k * F_CORE : (k + 1) * F_CORE] = res.results[k]["outT"].T.astype(
            np.float32
        )
    return out

